# revision 30
# baseline (speedup 1.0000x reference)
"""Causal self-attention (B=4, T=2048, C=1024, H=16) on 8 TRN2 NeuronCores.

Sharding: core = (batch b, head-group g) with b = core//2, g = core%2.
Each core computes, for its batch and its 8 heads:
  QKV projection (W_qkv column shard), causal attention, and a PARTIAL
  output projection (W_pr row shard).  Host sums the two partials per
  batch and adds b_pr.

On-chip layout (per core):
  xT   [C, T]   : x[b].T               (DMA'd per 512-wide t-chunk)
  Q^T/K^T [512, T] : computed transposed (lhsT = W slice, rhs = xT)
  V    [T, 512] : computed natural     (lhsT = xT slice,  rhs = Wv)
  Attention per chunk j (q in [512j, 512j+512)):
    S^T blocks [k-tile 128, q 512] = K_h^T.T-free matmul (contraction d=64)
    exp on ACT (scale=1/8) -> A^T bf16, causal mask via gpsimd.affine_select
    Y^T[65, q] += [V_h | ones].T @ A^T   (row 64 = softmax denominators)
    normalize with reciprocal_approx_accurate + gpsimd.partition_broadcast
  Projection: out^T[c_out, t] += W_pr_shard.T @ Y^T  (fp32r)
"""

import numpy as np

import concourse.bass as bass
import concourse.mybir as mybir
import concourse.tile as tile
from concourse.bass_utils import run_bass_kernel_spmd


def _split_multiwaits(nc: bass.Bass, max_waits: int = 1) -> None:
    """The walrus build in this container rejects >max_waits sync-waits on an
    instruction ("Too many sync wait commands").  Move extra waits onto
    same-engine NoOps inserted immediately before the instruction — the
    engine blocks on each NoOp's wait first, so semantics are unchanged."""
    n = 0
    for fn in nc.m.functions:
        for blk in fn.blocks:
            out = []
            for inst in blk.instructions:
                si = getattr(inst, "sync_info", None)
                waits = list(si.on_wait) if si is not None and si.on_wait else []
                if len(waits) > max_waits:
                    keep = waits[-max_waits:]
                    for w in waits[: -max_waits]:
                        nop = mybir.InstNoOp(name=f"{inst.name}-w{n}", ins=[], outs=[])
                        n += 1
                        nop.engine = inst.engine
                        nop.sync_info = mybir.SyncInfo(on_wait=[w], on_update=[])
                        out.append(nop)
                    inst.sync_info = mybir.SyncInfo(
                        on_wait=keep, on_update=list(si.on_update or [])
                    )
                out.append(inst)
            blk.instructions = out

AF = mybir.ActivationFunctionType
ALU = mybir.AluOpType

F32 = mybir.dt.float32
F32R = mybir.dt.float32r
BF16 = mybir.dt.bfloat16

B, T_FULL, C = 4, 2048, 1024
H, HD = 16, 64
HPC = 8              # heads per core
GC = HPC * HD        # 512: per-core head-group width
P = 128
CH = 512             # q-chunk width
NKC = C // P         # 8 k-tiles over the C contraction

NP_BF16 = mybir.dt.np(BF16)


def build_attention(T: int = T_FULL, split_waits: bool = True) -> bass.Bass:
    assert T % CH == 0
    nch = T // CH        # q-chunks
    ntt = T // P         # t-tiles

    nc = bass.Bass("TRN2", debug=False, num_devices=8)

    xT_d = nc.dram_tensor("xT", [C, T], BF16, kind="ExternalInput").ap()
    wq_d = nc.dram_tensor("wq", [C, GC], BF16, kind="ExternalInput").ap()
    wk_d = nc.dram_tensor("wk", [C, GC], BF16, kind="ExternalInput").ap()
    wv_d = nc.dram_tensor("wv", [C, GC], BF16, kind="ExternalInput").ap()
    bq_d = nc.dram_tensor("bq", [GC], F32, kind="ExternalInput").ap()
    bk_d = nc.dram_tensor("bk", [GC], F32, kind="ExternalInput").ap()
    bv_d = nc.dram_tensor("bv", [GC], BF16, kind="ExternalInput").ap()
    wpr_d = nc.dram_tensor("wpr", [GC, C], BF16, kind="ExternalInput").ap()
    out_d = nc.dram_tensor("outT", [C, T], F32, kind="ExternalOutput").ap()

    with tile.TileContext(nc) as tc:
        with (
            tc.tile_pool(name="singles", bufs=1) as singles,
            tc.tile_pool(name="xt", bufs=2) as xt_pool,
            tc.tile_pool(name="qt", bufs=2) as qt_pool,
            tc.tile_pool(name="at", bufs=3) as at_pool,
            tc.tile_pool(name="yt", bufs=2) as yt_pool,
            tc.tile_pool(name="dd", bufs=3) as d_pool,
            tc.tile_pool(name="bc", bufs=3) as bc_pool,
            tc.tile_pool(name="ot", bufs=2) as out_pool,
            tc.tile_pool(name="swps", bufs=2, space="PSUM") as sweep_ps,
            tc.tile_pool(name="scps", bufs=2, space="PSUM") as sc_ps,
            tc.tile_pool(name="yps", bufs=1, space="PSUM") as y_ps,
            tc.tile_pool(name="pjps", bufs=1, space="PSUM") as proj_ps,
        ):
            # ---- resident tensors ----
            wq_sb = singles.tile([P, NKC, GC], BF16)
            wk_sb = singles.tile([P, NKC, GC], BF16)
            wv_sb = singles.tile([P, NKC, GC], BF16)
            nc.sync.dma_start(out=wq_sb, in_=wq_d.rearrange("(kc p) n -> p kc n", p=P))
            nc.sync.dma_start(out=wk_sb, in_=wk_d.rearrange("(kc p) n -> p kc n", p=P))
            nc.sync.dma_start(out=wv_sb, in_=wv_d.rearrange("(kc p) n -> p kc n", p=P))
            wpr_sb = singles.tile([P, GC // P, C], BF16)
            nc.sync.dma_start(
                out=wpr_sb, in_=wpr_d.rearrange("(kp p) m -> p kp m", p=P)
            )
            bqk_sb = singles.tile([P, 2, GC // P], F32)
            nc.sync.dma_start(
                out=bqk_sb[:, 0, :], in_=bq_d.rearrange("(m p) -> p m", p=P)
            )
            nc.sync.dma_start(
                out=bqk_sb[:, 1, :], in_=bk_d.rearrange("(m p) -> p m", p=P)
            )
            bv_sb = singles.tile([1, GC], BF16)
            nc.sync.dma_start(out=bv_sb, in_=bv_d.rearrange("(o n) -> o n", o=1))
            ones_sb = singles.tile([1, P], BF16)
            nc.vector.memset(ones_sb, 1.0)
            ones64_sb = singles.tile([P, HD], BF16)
            nc.vector.memset(ones64_sb, 1.0)

            kt_sb = singles.tile([P, GC // P, T], BF16)       # K^T, filled per chunk
            v_sb = singles.tile([P, ntt, HPC, HD + 1], BF16)  # [V | 1] per t-tile/head
            nc.vector.memset(v_sb[:, :, :, HD : HD + 1], 1.0)

            for j in range(nch):
                nkt = 4 * (j + 1)           # k-tiles valid for this q-chunk
                tsl = slice(j * CH, (j + 1) * CH)

                # ---- sweep: QT/KT chunk-j columns, V t-tiles 4j..4j+3 ----
                xt_t = xt_pool.tile([P, NKC, CH], BF16)
                nc.sync.dma_start(
                    out=xt_t, in_=xT_d[:, tsl].rearrange("(kc p) t -> p kc t", p=P)
                )

                qt_t = qt_pool.tile([P, GC // P, CH], BF16)
                for mq in range(GC // P):
                    ps = sweep_ps.tile([P, CH], F32)
                    for kc in range(NKC):
                        nc.tensor.matmul(
                            ps,
                            lhsT=wq_sb[:, kc, mq * P : (mq + 1) * P],
                            rhs=xt_t[:, kc, :],
                            start=(kc == 0),
                            stop=(kc == NKC - 1),
                        )
                    nc.vector.tensor_scalar_add(
                        out=qt_t[:, mq, :], in0=ps, scalar1=bqk_sb[:, 0, mq : mq + 1]
                    )
                for mk in range(GC // P):
                    ps = sweep_ps.tile([P, CH], F32)
                    for kc in range(NKC):
                        nc.tensor.matmul(
                            ps,
                            lhsT=wk_sb[:, kc, mk * P : (mk + 1) * P],
                            rhs=xt_t[:, kc, :],
                            start=(kc == 0),
                            stop=(kc == NKC - 1),
                        )
                    nc.vector.tensor_scalar_add(
                        out=kt_sb[:, mk, tsl], in0=ps, scalar1=bqk_sb[:, 1, mk : mk + 1]
                    )
                for tl in range(4):
                    tt = 4 * j + tl
                    ps = sweep_ps.tile([P, GC], F32)
                    for kc in range(NKC):
                        nc.tensor.matmul(
                            ps,
                            lhsT=xt_t[:, kc, tl * P : (tl + 1) * P],
                            rhs=wv_sb[:, kc, :],
                            start=(kc == 0),
                            stop=False,
                        )
                    # bias row: V += ones.T @ bv
                    nc.tensor.matmul(
                        ps, lhsT=ones_sb, rhs=bv_sb, start=False, stop=True
                    )
                    nc.vector.tensor_copy(
                        v_sb[:, tt, :, 0:HD], ps.rearrange("p (h d) -> p h d", h=HPC)
                    )

                # ---- attention for chunk j ----
                yt_t = yt_pool.tile([P, GC // P, CH], BF16)
                for h in range(HPC):
                    mk, po = h // 2, HD * (h % 2)
                    yps = y_ps.tile([P, CH], F32)  # rows 0..64 used
                    for pr in range(nkt // 2):
                        sc = sc_ps.tile([P, 2 * CH], F32)
                        for u in range(2):
                            ki = 2 * pr + u
                            nc.tensor.matmul(
                                sc[:, u * CH : (u + 1) * CH],
                                lhsT=kt_sb[po : po + HD, mk, ki * P : (ki + 1) * P],
                                rhs=qt_t[po : po + HD, mk, :],
                                start=True,
                                stop=True,
                            )
                        at_t = at_pool.tile([P, 2 * CH], BF16)
                        nc.scalar.activation(at_t, sc, AF.Exp, scale=0.125)
                        for u in range(2):
                            ki = 2 * pr + u
                            r = ki - 4 * j
                            if r >= 0:  # diagonal block: zero where q < k
                                blk = at_t[:, u * CH : (u + 1) * CH]
                                nc.gpsimd.affine_select(
                                    out=blk,
                                    in_=blk,
                                    pattern=[[1, CH]],
                                    compare_op=ALU.is_ge,
                                    fill=0.0,
                                    base=-P * r,
                                    channel_multiplier=-1,
                                )
                        for u in range(2):
                            ki = 2 * pr + u
                            nc.tensor.matmul(
                                yps[0 : HD + 1, :],
                                lhsT=v_sb[:, ki, h, :],
                                rhs=at_t[:, u * CH : (u + 1) * CH],
                                start=(ki == 0),
                                stop=(ki == nkt - 1),
                            )
                    # softmax denominator: Dinv = exp(-ln D) on ACT (custom-DVE
                    # reciprocal ops don't encode under this walrus build),
                    # broadcast across partitions via a rank-1 f32r matmul,
                    # then one fused PSUM-read * broadcast -> bf16 Y^T write.
                    dr_t = d_pool.tile([P, CH], F32)
                    dr2_t = d_pool.tile([P, CH], BF16, tag="dr2")
                    nc.scalar.activation(
                        dr_t[HD : HD + 1, :], yps[HD : HD + 1, :], AF.Ln
                    )
                    nc.scalar.activation(
                        dr2_t[HD : HD + 1, :],
                        dr_t[HD : HD + 1, :],
                        AF.Exp,
                        scale=-1.0,
                    )
                    bc_ps = proj_ps.tile([HD, CH], F32, tag="pp")
                    nc.tensor.matmul(
                        bc_ps,
                        lhsT=ones64_sb[HD : HD + 1, :],
                        rhs=dr2_t[HD : HD + 1, :],
                        start=True,
                        stop=True,
                    )
                    bc_t = bc_pool.tile([HD, CH], F32)
                    nc.vector.tensor_copy(bc_t, bc_ps)
                    nc.vector.tensor_mul(
                        yt_t[po : po + HD, mk, :], yps[0:HD, :], bc_t
                    )

                # ---- partial output projection for chunk j (bf16) ----
                for m in range(C // P):
                    pp = proj_ps.tile([P, CH], F32)
                    for kp in range(GC // P):
                        nc.tensor.matmul(
                            pp,
                            lhsT=wpr_sb[:, kp, m * P : (m + 1) * P],
                            rhs=yt_t[:, kp, :],
                            start=(kp == 0),
                            stop=(kp == GC // P - 1),
                        )
                    ot = out_pool.tile([P, CH], F32)
                    nc.scalar.copy(ot, pp)
                    nc.sync.dma_start(out=out_d[m * P : (m + 1) * P, tsl], in_=ot)

    if split_waits:  # breaks CoreSim's sem bookkeeping; needed for walrus
        _split_multiwaits(nc)
    return nc


def make_in_maps(x, W_qkv, b_qkv, W_pr):
    """Shard FULL inputs into the 8 per-core input dicts."""
    x = np.asarray(x, dtype=np.float32)
    W_qkv = np.asarray(W_qkv, dtype=np.float32)
    b_qkv = np.asarray(b_qkv, dtype=np.float32)
    W_pr = np.asarray(W_pr, dtype=np.float32)
    in_maps = []
    for core in range(8):
        b, g = divmod(core, 2)
        sl = slice(g * GC, (g + 1) * GC)
        in_maps.append(
            {
                "xT": np.ascontiguousarray(x[b].T).astype(NP_BF16),
                "wq": np.ascontiguousarray(W_qkv[:, 0 * C :][:, sl]).astype(NP_BF16),
                "wk": np.ascontiguousarray(W_qkv[:, 1 * C :][:, sl]).astype(NP_BF16),
                "wv": np.ascontiguousarray(W_qkv[:, 2 * C :][:, sl]).astype(NP_BF16),
                "bq": np.ascontiguousarray(b_qkv[0 * C :][sl]),
                "bk": np.ascontiguousarray(b_qkv[1 * C :][sl]),
                "bv": np.ascontiguousarray(b_qkv[2 * C :][sl]).astype(NP_BF16),
                "wpr": np.ascontiguousarray(W_pr[sl, :]).astype(NP_BF16),
            }
        )
    return in_maps


def assemble_output(parts, b_pr):
    """parts: 8 per-core outT [C, T] partials -> full [B, T, C] output."""
    b_pr = np.asarray(b_pr, dtype=np.float32)
    out = np.empty((B, T_FULL, C), dtype=np.float32)
    for b in range(B):
        out[b] = (parts[2 * b] + parts[2 * b + 1]).T + b_pr
    return out


_CACHE = {}


def kernel(x, W_qkv, b_qkv, W_pr, b_pr):
    if "nc" not in _CACHE:
        _CACHE["nc"] = build_attention(T_FULL)
    in_maps = make_in_maps(x, W_qkv, b_qkv, W_pr)
    res = run_bass_kernel_spmd(_CACHE["nc"], in_maps, core_ids=list(range(8)))
    parts = [r["outT"] for r in res.results]
    return assemble_output(parts, b_pr)


# revision 33
# speedup vs baseline: 1.0331x; 1.0331x over previous
"""Causal self-attention (B=4, T=2048, C=1024, H=16) on 8 TRN2 NeuronCores.

Sharding: core = (batch b, head-group g) with b = core//2, g = core%2.
Each core computes, for its batch and its 8 heads:
  QKV projection (W_qkv column shard), causal attention, and a PARTIAL
  output projection (W_pr row shard).  Host sums the two partials per
  batch and adds b_pr.

On-chip layout (per core):
  xT   [C, T]   : x[b].T               (DMA'd per 512-wide t-chunk)
  Q^T/K^T [512, T] : computed transposed (lhsT = W slice, rhs = xT)
  V    [T, 512] : computed natural     (lhsT = xT slice,  rhs = Wv)
  Attention per chunk j (q in [512j, 512j+512)):
    S^T blocks [k-tile 128, q 512] = K_h^T.T-free matmul (contraction d=64)
    exp on ACT (scale=1/8) -> A^T bf16, causal mask via gpsimd.affine_select
    Y^T[65, q] += [V_h | ones].T @ A^T   (row 64 = softmax denominators)
    normalize with reciprocal_approx_accurate + gpsimd.partition_broadcast
  Projection: out^T[c_out, t] += W_pr_shard.T @ Y^T  (fp32r)
"""

import numpy as np

import concourse.bass as bass
import concourse.mybir as mybir
import concourse.tile as tile
from concourse.bass_utils import run_bass_kernel_spmd


def _split_multiwaits(nc: bass.Bass, max_waits: int = 1) -> None:
    """The walrus build in this container rejects >max_waits sync-waits on an
    instruction ("Too many sync wait commands").  Move extra waits onto
    same-engine NoOps inserted immediately before the instruction — the
    engine blocks on each NoOp's wait first, so semantics are unchanged."""
    n = 0
    for fn in nc.m.functions:
        for blk in fn.blocks:
            out = []
            for inst in blk.instructions:
                si = getattr(inst, "sync_info", None)
                waits = list(si.on_wait) if si is not None and si.on_wait else []
                if len(waits) > max_waits:
                    keep = waits[-max_waits:]
                    for w in waits[: -max_waits]:
                        nop = mybir.InstNoOp(name=f"{inst.name}-w{n}", ins=[], outs=[])
                        n += 1
                        nop.engine = inst.engine
                        nop.sync_info = mybir.SyncInfo(on_wait=[w], on_update=[])
                        out.append(nop)
                    inst.sync_info = mybir.SyncInfo(
                        on_wait=keep, on_update=list(si.on_update or [])
                    )
                out.append(inst)
            blk.instructions = out

AF = mybir.ActivationFunctionType
ALU = mybir.AluOpType

F32 = mybir.dt.float32
F32R = mybir.dt.float32r
BF16 = mybir.dt.bfloat16

B, T_FULL, C = 4, 2048, 1024
H, HD = 16, 64
HPC = 8              # heads per core
GC = HPC * HD        # 512: per-core head-group width
P = 128
CH = 512             # q-chunk width
NKC = C // P         # 8 k-tiles over the C contraction

NP_BF16 = mybir.dt.np(BF16)


def build_attention(T: int = T_FULL, split_waits: bool = True) -> bass.Bass:
    assert T % CH == 0
    nch = T // CH        # q-chunks
    ntt = T // P         # t-tiles

    nc = bass.Bass("TRN2", debug=False, num_devices=8)

    xT_d = nc.dram_tensor("xT", [C, T], BF16, kind="ExternalInput").ap()
    wq_d = nc.dram_tensor("wq", [C, GC], BF16, kind="ExternalInput").ap()
    wk_d = nc.dram_tensor("wk", [C, GC], BF16, kind="ExternalInput").ap()
    wv_d = nc.dram_tensor("wv", [C, GC], BF16, kind="ExternalInput").ap()
    bq_d = nc.dram_tensor("bq", [GC], F32, kind="ExternalInput").ap()
    bk_d = nc.dram_tensor("bk", [GC], F32, kind="ExternalInput").ap()
    bv_d = nc.dram_tensor("bv", [GC], BF16, kind="ExternalInput").ap()
    wpr_d = nc.dram_tensor("wpr", [GC, C], BF16, kind="ExternalInput").ap()
    out_d = nc.dram_tensor("outT", [C, T], F32, kind="ExternalOutput").ap()

    with tile.TileContext(nc) as tc:
        with (
            tc.tile_pool(name="singles", bufs=1) as singles,
            tc.tile_pool(name="xt", bufs=2) as xt_pool,
            tc.tile_pool(name="qt", bufs=2) as qt_pool,
            tc.tile_pool(name="at", bufs=3) as at_pool,
            tc.tile_pool(name="yt", bufs=2) as yt_pool,
            tc.tile_pool(name="dd", bufs=3) as d_pool,
            tc.tile_pool(name="bc", bufs=3) as bc_pool,
            tc.tile_pool(name="ot", bufs=2) as out_pool,
            tc.tile_pool(name="swps", bufs=2, space="PSUM") as sweep_ps,
            tc.tile_pool(name="scps", bufs=2, space="PSUM") as sc_ps,
            tc.tile_pool(name="yps", bufs=2, space="PSUM") as y_ps,
        ):
            # ---- resident tensors ----
            wq_sb = singles.tile([P, NKC, GC], BF16)
            wk_sb = singles.tile([P, NKC, GC], BF16)
            wv_sb = singles.tile([P, NKC, GC], BF16)
            nc.sync.dma_start(out=wq_sb, in_=wq_d.rearrange("(kc p) n -> p kc n", p=P))
            nc.sync.dma_start(out=wk_sb, in_=wk_d.rearrange("(kc p) n -> p kc n", p=P))
            nc.sync.dma_start(out=wv_sb, in_=wv_d.rearrange("(kc p) n -> p kc n", p=P))
            wpr_sb = singles.tile([P, GC // P, C], BF16)
            nc.sync.dma_start(
                out=wpr_sb, in_=wpr_d.rearrange("(kp p) m -> p kp m", p=P)
            )
            bqk_sb = singles.tile([P, 2, GC // P], F32)
            nc.sync.dma_start(
                out=bqk_sb[:, 0, :], in_=bq_d.rearrange("(m p) -> p m", p=P)
            )
            nc.sync.dma_start(
                out=bqk_sb[:, 1, :], in_=bk_d.rearrange("(m p) -> p m", p=P)
            )
            bv_sb = singles.tile([1, GC], BF16)
            nc.sync.dma_start(out=bv_sb, in_=bv_d.rearrange("(o n) -> o n", o=1))
            ones_sb = singles.tile([1, P], BF16)
            nc.vector.memset(ones_sb, 1.0)
            ones64_sb = singles.tile([P, HD], BF16)
            nc.vector.memset(ones64_sb, 1.0)

            kt_sb = singles.tile([P, GC // P, T], BF16)       # K^T, filled per chunk
            v_sb = singles.tile([P, ntt, HPC, HD + 1], BF16)  # [V | 1] per t-tile/head
            nc.vector.memset(v_sb[:, :, :, HD : HD + 1], 1.0)

            for j in range(nch):
                nkt = 4 * (j + 1)           # k-tiles valid for this q-chunk
                tsl = slice(j * CH, (j + 1) * CH)

                # ---- sweep: QT/KT chunk-j columns, V t-tiles 4j..4j+3 ----
                xt_t = xt_pool.tile([P, NKC, CH], BF16)
                nc.sync.dma_start(
                    out=xt_t, in_=xT_d[:, tsl].rearrange("(kc p) t -> p kc t", p=P)
                )

                qt_t = qt_pool.tile([P, GC // P, CH], BF16)
                for mq in range(GC // P):
                    ps = sweep_ps.tile([P, CH], F32)
                    for kc in range(NKC):
                        nc.tensor.matmul(
                            ps,
                            lhsT=wq_sb[:, kc, mq * P : (mq + 1) * P],
                            rhs=xt_t[:, kc, :],
                            start=(kc == 0),
                            stop=(kc == NKC - 1),
                        )
                    nc.vector.tensor_scalar_add(
                        out=qt_t[:, mq, :], in0=ps, scalar1=bqk_sb[:, 0, mq : mq + 1]
                    )
                for mk in range(GC // P):
                    ps = sweep_ps.tile([P, CH], F32)
                    for kc in range(NKC):
                        nc.tensor.matmul(
                            ps,
                            lhsT=wk_sb[:, kc, mk * P : (mk + 1) * P],
                            rhs=xt_t[:, kc, :],
                            start=(kc == 0),
                            stop=(kc == NKC - 1),
                        )
                    nc.vector.tensor_scalar_add(
                        out=kt_sb[:, mk, tsl], in0=ps, scalar1=bqk_sb[:, 1, mk : mk + 1]
                    )
                for tl in range(4):
                    tt = 4 * j + tl
                    ps = sweep_ps.tile([P, GC], F32)
                    for kc in range(NKC):
                        nc.tensor.matmul(
                            ps,
                            lhsT=xt_t[:, kc, tl * P : (tl + 1) * P],
                            rhs=wv_sb[:, kc, :],
                            start=(kc == 0),
                            stop=False,
                        )
                    # bias row: V += ones.T @ bv
                    nc.tensor.matmul(
                        ps, lhsT=ones_sb, rhs=bv_sb, start=False, stop=True
                    )
                    nc.vector.tensor_copy(
                        v_sb[:, tt, :, 0:HD], ps.rearrange("p (h d) -> p h d", h=HPC)
                    )

                # ---- attention for chunk j: head pairs interleaved so the
                # two K=64 scores matmuls land in different PE row groups
                # (base partitions 0/64) and run concurrently, with the next
                # LDWEIGHTS pulled ahead by the PE reorder window. ----
                yt_t = yt_pool.tile([P, GC // P, CH], BF16)
                for hp in range(HPC // 2):
                    mk = hp
                    yps0 = y_ps.tile([P, CH], F32, tag="yps")  # rows 0..64
                    yps1 = y_ps.tile([P, CH], F32, tag="yps")
                    ypss = [yps0, yps1]
                    for ki in range(nkt):
                        sc = sc_ps.tile([P, 2 * CH], F32)
                        for u in range(2):  # u = head parity; po = 64*u
                            po = HD * u
                            nc.tensor.matmul(
                                sc[:, u * CH : (u + 1) * CH],
                                lhsT=kt_sb[po : po + HD, mk, ki * P : (ki + 1) * P],
                                rhs=qt_t[po : po + HD, mk, :],
                                start=True,
                                stop=True,
                            )
                        at_t = at_pool.tile([P, 2 * CH], BF16)
                        nc.scalar.activation(at_t, sc, AF.Exp, scale=0.125)
                        r = ki - 4 * j
                        if r >= 0:  # diagonal block: zero where q < k
                            for u in range(2):
                                blk = at_t[:, u * CH : (u + 1) * CH]
                                nc.gpsimd.affine_select(
                                    out=blk,
                                    in_=blk,
                                    pattern=[[1, CH]],
                                    compare_op=ALU.is_ge,
                                    fill=0.0,
                                    base=-P * r,
                                    channel_multiplier=-1,
                                )
                        for u in range(2):
                            nc.tensor.matmul(
                                ypss[u][0 : HD + 1, :],
                                lhsT=v_sb[:, ki, 2 * hp + u, :],
                                rhs=at_t[:, u * CH : (u + 1) * CH],
                                start=(ki == 0),
                                stop=(ki == nkt - 1),
                            )
                    for u in range(2):
                        po = HD * u
                        yps = ypss[u]
                        # softmax denominator: Dinv = exp(-ln D) on ACT
                        # (custom-DVE reciprocal doesn't encode under this
                        # walrus build), broadcast across partitions via a
                        # rank-1 bf16 matmul, then one fused
                        # PSUM-read * broadcast -> bf16 Y^T write on DVE.
                        dr_t = d_pool.tile([P, CH], F32)
                        dr2_t = d_pool.tile([P, CH], BF16, tag="dr2")
                        nc.scalar.activation(
                            dr_t[HD : HD + 1, :], yps[HD : HD + 1, :], AF.Ln
                        )
                        nc.scalar.activation(
                            dr2_t[HD : HD + 1, :],
                            dr_t[HD : HD + 1, :],
                            AF.Exp,
                            scale=-1.0,
                        )
                        bc_ps = sweep_ps.tile([HD, CH], F32, tag="ps")
                        nc.tensor.matmul(
                            bc_ps,
                            lhsT=ones64_sb[HD : HD + 1, :],
                            rhs=dr2_t[HD : HD + 1, :],
                            start=True,
                            stop=True,
                        )
                        bc_t = bc_pool.tile([HD, CH], F32)
                        nc.vector.tensor_copy(bc_t, bc_ps)
                        nc.vector.tensor_mul(
                            yt_t[po : po + HD, mk, :], yps[0:HD, :], bc_t
                        )

                # ---- partial output projection for chunk j (bf16) ----
                for m in range(C // P):
                    pp = sweep_ps.tile([P, CH], F32, tag="ps")
                    for kp in range(GC // P):
                        nc.tensor.matmul(
                            pp,
                            lhsT=wpr_sb[:, kp, m * P : (m + 1) * P],
                            rhs=yt_t[:, kp, :],
                            start=(kp == 0),
                            stop=(kp == GC // P - 1),
                        )
                    ot = out_pool.tile([P, CH], F32)
                    nc.vector.tensor_copy(ot, pp)
                    nc.sync.dma_start(out=out_d[m * P : (m + 1) * P, tsl], in_=ot)

    if split_waits:  # breaks CoreSim's sem bookkeeping; needed for walrus
        _split_multiwaits(nc)
    return nc


def make_in_maps(x, W_qkv, b_qkv, W_pr):
    """Shard FULL inputs into the 8 per-core input dicts."""
    x = np.asarray(x, dtype=np.float32)
    W_qkv = np.asarray(W_qkv, dtype=np.float32)
    b_qkv = np.asarray(b_qkv, dtype=np.float32)
    W_pr = np.asarray(W_pr, dtype=np.float32)
    in_maps = []
    for core in range(8):
        b, g = divmod(core, 2)
        sl = slice(g * GC, (g + 1) * GC)
        in_maps.append(
            {
                "xT": np.ascontiguousarray(x[b].T).astype(NP_BF16),
                "wq": np.ascontiguousarray(W_qkv[:, 0 * C :][:, sl]).astype(NP_BF16),
                "wk": np.ascontiguousarray(W_qkv[:, 1 * C :][:, sl]).astype(NP_BF16),
                "wv": np.ascontiguousarray(W_qkv[:, 2 * C :][:, sl]).astype(NP_BF16),
                "bq": np.ascontiguousarray(b_qkv[0 * C :][sl]),
                "bk": np.ascontiguousarray(b_qkv[1 * C :][sl]),
                "bv": np.ascontiguousarray(b_qkv[2 * C :][sl]).astype(NP_BF16),
                "wpr": np.ascontiguousarray(W_pr[sl, :]).astype(NP_BF16),
            }
        )
    return in_maps


def assemble_output(parts, b_pr):
    """parts: 8 per-core outT [C, T] partials -> full [B, T, C] output."""
    b_pr = np.asarray(b_pr, dtype=np.float32)
    out = np.empty((B, T_FULL, C), dtype=np.float32)
    for b in range(B):
        out[b] = (parts[2 * b] + parts[2 * b + 1]).T + b_pr
    return out


_CACHE = {}


def kernel(x, W_qkv, b_qkv, W_pr, b_pr):
    if "nc" not in _CACHE:
        _CACHE["nc"] = build_attention(T_FULL)
    in_maps = make_in_maps(x, W_qkv, b_qkv, W_pr)
    res = run_bass_kernel_spmd(_CACHE["nc"], in_maps, core_ids=list(range(8)))
    parts = [r["outT"] for r in res.results]
    return assemble_output(parts, b_pr)
